# revision 1
# baseline (speedup 1.0000x reference)
"""Multi-head attention Trainium2 kernel (8 NeuronCores, SPMD).

Problem: nn_MultiHeadAttention (B=2, S=2048, D=768, H=12, d_k=64), f32 I/O.

Sharding: 24 (batch, head) pairs -> 8 cores x 3 heads. Core c handles
batch b = c // 4 and heads [3*(c%4), 3*(c%4)+3). Each core computes the
Q/K/V projections for its 3 heads, full-sequence attention, and its
partial contribution to the output projection. A 4-core ReduceScatter
(cores of the same batch) sums the partials and leaves each core with a
distinct 512-row slice of the batch output; the host concatenates.

On-device layouts are transposed (feature-major) so every matmul maps
directly onto the PE array (out = lhsT.T @ rhs, contraction on the
partition dim):
  - q/k/v are shipped as qT/kT/vT [768, S] bf16 (host transpose + cast)
  - weights shipped pre-transposed/sliced; softmax runs on transposed
    scores sT[kv, q] = K_h Q_h^T so attn @ V becomes V^T @ eT with
    natural-layout V as the stationary operand
  - softmax skips max-subtraction (scores are O(1) for this problem) and
    gets the denominator for free from a ones-column appended to V
  - output bias enters via a ones-row appended to the outT stack
"""

import numpy as np
import ml_dtypes

B = 2
S = 2048
D = 768
H = 12
DK = 64
HPC = 3           # heads per core
HD = HPC * DK     # 192 head-feature columns per core
NCORES = 8
GROUP = 4         # cores per batch (reduce-scatter group)
QS = S // GROUP   # 512 output rows per core

_compiled = None


def _build(reps=1, collective=True):
    """Build the SPMD program. reps>1 emits the whole pipeline N times
    back-to-back (same inputs/outputs) — used only for timing, where
    (T_reps - T_1)/(reps-1) cancels the per-dispatch overhead.
    collective=False drops the final ReduceScatter (for TimelineSim)."""
    import concourse.mybir as mybir
    import concourse.tile as tile
    from concourse import bacc
    from concourse.bass import ts

    bf16 = mybir.dt.bfloat16
    f32 = mybir.dt.float32

    nc = bacc.Bacc(num_devices=NCORES)

    qt = nc.dram_tensor("qt", [D, S], bf16, kind="ExternalInput")
    kt = nc.dram_tensor("kt", [D, S], bf16, kind="ExternalInput")
    vt = nc.dram_tensor("vt", [D, S], bf16, kind="ExternalInput")
    wq = nc.dram_tensor("wq", [D, HD], bf16, kind="ExternalInput")
    wk = nc.dram_tensor("wk", [D, HD], bf16, kind="ExternalInput")
    wv = nc.dram_tensor("wv", [D, HD], bf16, kind="ExternalInput")
    wo = nc.dram_tensor("wo", [HD + 1, D], bf16, kind="ExternalInput")
    bq = nc.dram_tensor("bq", [HD, 1], f32, kind="ExternalInput")
    bk = nc.dram_tensor("bk", [HD, 1], f32, kind="ExternalInput")
    bv = nc.dram_tensor("bv", [1, HD], f32, kind="ExternalInput")
    out_ext = nc.dram_tensor("out", [QS, D], bf16, kind="ExternalOutput")
    out_part = nc.dram_tensor("out_part", [S, D], bf16)
    out_rs = nc.dram_tensor("out_rs", [QS, D], bf16)
    # DRAM bounce rows for broadcasting softmax reciprocals across
    # partitions (SBUF->SBUF partition-broadcast DMA is not allowed).
    rscratch = nc.dram_tensor("rscratch", [HPC * (S // 512), 512], f32)

    RGROUPS = [list(range(g * GROUP, (g + 1) * GROUP))
               for g in range(NCORES // GROUP)]
    NC_ = D // 128      # 6 contraction chunks for the projections
    NKC = S // 128      # 16 kv chunks
    NQB = S // 512      # 4 q blocks
    VW = DK + 2         # 66-wide per-head V block: 64 dims + ones col + pad

    import contextlib

    with tile.TileContext(nc) as tc:
      with (tc.For_i(0, reps, 1) if reps > 1 else contextlib.nullcontext()):
       with contextlib.ExitStack() as ctx:
        sfx = ""
        consts = ctx.enter_context(tc.tile_pool(name="consts" + sfx, bufs=1))
        acts = ctx.enter_context(tc.tile_pool(name="acts" + sfx, bufs=1))

        # ---- load inputs, in consumption order ----
        # DMA bandwidth is the startup bound, so emit transfers in the
        # order compute consumes them (wq -> qt -> wk -> kt -> wv -> vt
        # -> wo), alternating the two HWDGE queues (SP + ACT).
        import concourse.bass as bass
        dmae = [nc.sync, nc.scalar]
        ins_sb, w_sb, bias_sb = {}, {}, {}

        def load_w(name, t):
            sb = consts.tile([128, NC_, HD], bf16, tag=name)
            nc.scalar.dma_start(
                out=sb, in_=t[:, :].rearrange("(c p) n -> p c n", p=128))
            w_sb[name] = sb

        def load_bias(name, t):
            b0 = consts.tile([128, 1], f32, tag=name + "0")
            nc.sync.dma_start(out=b0, in_=t[0:128, :])
            b1 = consts.tile([HD - 128, 1], f32, tag=name + "1")
            nc.sync.dma_start(out=b1, in_=t[128:HD, :])
            bias_sb[name] = (b0, b1)

        def load_in(name, t, di=[0]):
            sb = consts.tile([128, NC_, S], bf16, tag=name)
            for c in range(NC_):
                dmae[di[0] % 2].dma_start(
                    out=sb[:, c, :], in_=t[c * 128:(c + 1) * 128, :])
                di[0] += 1
            ins_sb[name] = sb

        load_w("wk", wk)
        load_bias("bk", bk)
        load_in("kt", kt)
        load_w("wq", wq)
        load_bias("bq", bq)
        load_in("qt", qt)
        load_w("wv", wv)
        bv_bc = consts.tile([128, HD], f32, tag="bv")
        nc.sync.dma_start(
            out=bv_bc,
            in_=bass.AP(tensor=bv[:, :].tensor, offset=bv[:, :].offset,
                        ap=[[0, 128]] + bv[:, :].ap[1:]))
        load_in("vt", vt)
        wo0 = consts.tile([128, D], bf16, tag="wo0")
        nc.scalar.dma_start(out=wo0, in_=wo[0:128, :])
        wo1 = consts.tile([HD + 1 - 128, D], bf16, tag="wo1")
        nc.scalar.dma_start(out=wo1, in_=wo[128:HD + 1, :])
        # Touch the exp table early so ACT's table DMA overlaps the loads.
        warm = consts.tile([1, 1], f32, tag="warm")
        nc.vector.memset(warm, 0.0)
        nc.scalar.activation(out=warm, in_=warm,
                             func=mybir.ActivationFunctionType.Exp)

        # ---- Q/K projections into transposed per-head-group layout ----
        # group 0: heads 0,1 stacked on partitions 0..127; group 1: head 2.
        # The projection accumulators share the scores PSUM pool (same
        # tag) so the attention phase isn't gated on a pool-close
        # boundary: sc 2x3 banks + pv 2x1 = all 8 PSUM banks, one pool
        # lifetime across both phases.
        GRPS = [(0, 128), (128, 64)]
        proj = {}
        # PSUM budget: scores 2x3 banks (exclusively theirs, so attention
        # never waits on projection slot rotation) + 2x1-bank accumulators
        # shared in time by Q/K/V projections and PV = 8 banks.
        with tc.tile_pool(name="sc_psum" + sfx, bufs=2, space="PSUM") as sc_psum, \
                tc.tile_pool(name="acc_psum" + sfx, bufs=2, space="PSUM") as acc_psum:
            def emit_qk_proj(gi):
                off, m = GRPS[gi]
                for name in ("q", "k"):
                    dest = acts.tile([m, S], bf16, tag=f"{name}T{gi}")
                    proj[(name, gi)] = dest
                if gi == 0:
                    for name, wname, bname in (("k", "wk", "bk"),
                                               ("q", "wq", "bq")):
                        x_sb = ins_sb[name + "t"]
                        dest = proj[(name, gi)]
                        bias_ap = bias_sb[bname][gi]
                        for qb in range(NQB):
                            ps = acc_psum.tile([128, 512], f32, tag="acc")
                            for c in range(NC_):
                                nc.tensor.matmul(
                                    ps[0:m, :],
                                    lhsT=w_sb[wname][:, c, off:off + m],
                                    rhs=x_sb[:, c, ts(qb, 512)],
                                    start=(c == 0), stop=(c == NC_ - 1))
                            nc.vector.tensor_scalar_add(
                                out=dest[:, ts(qb, 512)], in0=ps[0:m, :],
                                scalar1=bias_ap[0:m, :])
                else:
                    # M=64 pair: Q-g1 on PE column-groups 0-1, K-g1 on 2-3,
                    # running concurrently in one accumulator tile.
                    for qb in range(NQB):
                        ps = acc_psum.tile([128, 512], f32, tag="acc")
                        for c in range(NC_):
                            nc.tensor.matmul(
                                ps[0:64, :],
                                lhsT=w_sb["wq"][:, c, off:off + 64],
                                rhs=ins_sb["qt"][:, c, ts(qb, 512)],
                                start=(c == 0), stop=(c == NC_ - 1),
                                tile_position=(0, 0))
                            nc.tensor.matmul(
                                ps[64:128, :],
                                lhsT=w_sb["wk"][:, c, off:off + 64],
                                rhs=ins_sb["kt"][:, c, ts(qb, 512)],
                                start=(c == 0), stop=(c == NC_ - 1),
                                tile_position=(0, 64))
                        nc.vector.tensor_scalar_add(
                            out=proj[("q", 1)][:, ts(qb, 512)],
                            in0=ps[0:64, :], scalar1=bias_sb["bq"][1])
                        nc.vector.tensor_scalar_add(
                            out=proj[("k", 1)][:, ts(qb, 512)],
                            in0=ps[64:128, :], scalar1=bias_sb["bk"][1])

            emit_qk_proj(0)

            # ---- attention helpers (emitted piecewise so PE/ACT have
            # scores work while vt still streams in) ----
            outT0 = acts.tile([128, S], bf16, tag="outT0")
            outT1 = acts.tile([DK + 1, S], bf16, tag="outT1")
            nc.vector.memset(outT1[DK:DK + 1, :], 1.0)
            v_sb = acts.tile([128, NKC, HPC * VW], bf16, tag="v")
            for h in range(HPC):
                nc.vector.memset(v_sb[:, :, h * VW + DK:h * VW + DK + 1], 1.0)
            # kc rounds of 3 (+1 tail): scores psum [128,3,512] double-buffered
            ROUNDS = [(0, 3), (3, 3), (6, 3), (9, 3), (12, 3), (15, 1)]
            sm_pool = ctx.enter_context(tc.tile_pool(name="sm" + sfx, bufs=2))
            nrm_pool = ctx.enter_context(tc.tile_pool(name="nrm" + sfx, bufs=4))

            def head_slices(h):
                if h < 2:
                    return (proj[("q", 0)][ts(h, 64), :],
                            proj[("k", 0)][ts(h, 64), :])
                return (proj[("q", 1)][0:64, :], proj[("k", 1)][0:64, :])

            def emit_scores(h, qb):
                qth, kth = head_slices(h)
                expt = sm_pool.tile([128, NKC, 512], bf16, tag="expt")
                for k0, klen in ROUNDS:
                    scps = sc_psum.tile([128, 3, 512], f32, tag="sc")
                    for j in range(klen):
                        nc.tensor.matmul(
                            scps[:, j, :],
                            lhsT=kth[:, ts(k0 + j, 128)],
                            rhs=qth[:, ts(qb, 512)],
                            start=True, stop=True)
                    nc.scalar.activation(
                        out=expt[:, k0:k0 + klen, :],
                        in_=scps[:, 0:klen, :],
                        func=mybir.ActivationFunctionType.Exp,
                        scale=float(1.0 / np.sqrt(DK)))
                return expt

            def emit_pv_norm(h, qb, expt):
                pvps = acc_psum.tile([DK + 1, 512], f32, tag="acc")
                for kc in range(NKC):
                    nc.tensor.matmul(
                        pvps,
                        lhsT=v_sb[:, kc, h * VW:h * VW + DK + 1],
                        rhs=expt[:, kc, :],
                        start=(kc == 0), stop=(kc == NKC - 1))
                recip = nrm_pool.tile([1, 512], f32, tag="recip")
                nc.vector.reciprocal(recip, pvps[DK:DK + 1, :])
                row = rscratch[h * NQB + qb:h * NQB + qb + 1, :]
                nc.sync.dma_start(out=row, in_=recip)
                rbc = nrm_pool.tile([64, 512], f32, tag="rbc")
                nc.sync.dma_start(
                    out=rbc,
                    in_=bass.AP(tensor=row.tensor, offset=row.offset,
                                ap=[[0, 64]] + row.ap[1:]))
                dst = (outT0[ts(h, 64), ts(qb, 512)] if h < 2
                       else outT1[0:64, ts(qb, 512)])
                nc.vector.tensor_mul(dst, pvps[0:DK, :], rbc)

            def emit_attn(h, skip=()):
                for qb in range(NQB):
                    if qb in skip:
                        continue
                    expt = emit_scores(h, qb)
                    emit_pv_norm(h, qb, expt)

            # scores for (h0, qb0..2) ahead of the V projection
            early = [emit_scores(0, 0), emit_scores(0, 1)]
            emit_qk_proj(1)

            # ---- V projection in natural layout, 66-stride head blocks ----
            for st in range(NKC):
                ps = acc_psum.tile([128, 512], f32, tag="acc")
                for c in range(NC_):
                    nc.tensor.matmul(
                        ps[:, 0:HD],
                        lhsT=ins_sb["vt"][:, c, ts(st, 128)],
                        rhs=w_sb["wv"][:, c, :],
                        start=(c == 0), stop=(c == NC_ - 1))
                for h in range(HPC):
                    nc.vector.tensor_add(
                        v_sb[:, st, h * VW:h * VW + DK],
                        ps[:, ts(h, 64)], bv_bc[:, ts(h, 64)])

            emit_pv_norm(0, 0, early[0])
            emit_pv_norm(0, 1, early[1])
            emit_attn(0, skip=(0, 1))
            emit_attn(1)
            emit_attn(2)

        # ---- output projection (bias via outT1 ones row x wo row 192) ----
        with tc.tile_pool(name="fo_psum" + sfx, bufs=2, space="PSUM") as fo_psum, \
                tc.tile_pool(name="fo" + sfx, bufs=2) as fo_pool:
            for og in range(4):            # 4 output groups of 4 q-tiles
                ot = fo_pool.tile([128, 4, D], bf16, tag="ot")
                for sq in range(4):
                    qt_ = og * 4 + sq
                    ps = fo_psum.tile([128, D], f32, tag="fo")
                    for noff, nsz in ((0, 512), (512, 256)):
                        nc.tensor.matmul(
                            ps[:, noff:noff + nsz],
                            lhsT=outT0[:, ts(qt_, 128)],
                            rhs=wo0[:, noff:noff + nsz],
                            start=True, stop=False)
                        nc.tensor.matmul(
                            ps[:, noff:noff + nsz],
                            lhsT=outT1[:, ts(qt_, 128)],
                            rhs=wo1[:, noff:noff + nsz],
                            start=False, stop=True)
                    nc.vector.tensor_copy(out=ot[:, sq, :], in_=ps)
                nc.sync.dma_start(
                    out=out_part[:, :].rearrange(
                        "(g t p) d -> g p t d", p=128, t=4)[og],
                    in_=ot)
                if collective:
                    nc.gpsimd.collective_compute(
                        "ReduceScatter", mybir.AluOpType.add,
                        replica_groups=RGROUPS,
                        ins=[out_part[ts(og, 512), :]],
                        outs=[out_rs[ts(og, 128), :]])
                nc.sync.dma_start(out=out_ext[ts(og, 128), :],
                                  in_=(out_rs if collective
                                       else out_part)[ts(og, 128), :])


    nc.compile()
    return nc


def _get_compiled():
    global _compiled
    if _compiled is None:
        _compiled = _build()
    return _compiled


def make_in_maps(q, k, v, Wq, bq, Wk, bk, Wv, bv, Wo, bo):
    bf = ml_dtypes.bfloat16
    in_maps = []
    for c in range(NCORES):
        b = c // GROUP
        g = c % GROUP
        cols = slice(g * HD, (g + 1) * HD)   # head-feature columns
        wo_aug = np.empty((HD + 1, D), np.float32)
        wo_aug[:HD] = Wo.T[cols.start:cols.stop, :]
        wo_aug[HD] = bo / GROUP              # summed GROUP times by the RS
        in_maps.append({
            "qt": np.ascontiguousarray(q[b].T).astype(bf),
            "kt": np.ascontiguousarray(k[b].T).astype(bf),
            "vt": np.ascontiguousarray(v[b].T).astype(bf),
            "wq": np.ascontiguousarray(Wq.T[:, cols]).astype(bf),
            "wk": np.ascontiguousarray(Wk.T[:, cols]).astype(bf),
            "wv": np.ascontiguousarray(Wv.T[:, cols]).astype(bf),
            "wo": wo_aug.astype(bf),
            "bq": np.ascontiguousarray(bq[cols].reshape(HD, 1)).astype(np.float32),
            "bk": np.ascontiguousarray(bk[cols].reshape(HD, 1)).astype(np.float32),
            "bv": np.ascontiguousarray(bv[cols].reshape(1, HD)).astype(np.float32),
        })
    return in_maps


def kernel(q, k, v, Wq, bq, Wk, bk, Wv, bv, Wo, bo):
    from concourse.bass_utils import run_bass_kernel_spmd

    q = np.asarray(q, np.float32)
    k = np.asarray(k, np.float32)
    v = np.asarray(v, np.float32)
    nc = _get_compiled()
    in_maps = make_in_maps(q, k, v,
                           np.asarray(Wq, np.float32), np.asarray(bq, np.float32),
                           np.asarray(Wk, np.float32), np.asarray(bk, np.float32),
                           np.asarray(Wv, np.float32), np.asarray(bv, np.float32),
                           np.asarray(Wo, np.float32), np.asarray(bo, np.float32))
    res = run_bass_kernel_spmd(nc, in_maps, list(range(NCORES))).results
    out = np.empty((B, S, D), np.float32)
    for c in range(NCORES):
        b = c // GROUP
        j = c % GROUP
        # chunked reduce-scatter: chunk g of core (b, j) holds batch-b
        # rows [512*g + 128*j, 512*g + 128*j + 128)
        chunks = res[c]["out"].reshape(GROUP, 128, D)
        for g in range(GROUP):
            out[b, 512 * g + 128 * j:512 * g + 128 * j + 128, :] = chunks[g]
    return out



# revision 7
# speedup vs baseline: 1.1189x; 1.1189x over previous
"""Multi-head attention Trainium2 kernel (8 NeuronCores, SPMD).

Problem: nn_MultiHeadAttention (B=2, S=2048, D=768, H=12, d_k=64), f32 I/O.

Sharding: 24 (batch, head) pairs -> 8 cores x 3 heads. Core c handles
batch b = c // 4 and heads [3*(c%4), 3*(c%4)+3). Each core computes the
Q/K/V projections for its 3 heads, full-sequence attention, and its
partial contribution to the output projection. A 4-core ReduceScatter
(cores of the same batch) sums the partials and leaves each core with a
distinct 512-row slice of the batch output; the host concatenates.

v2 schedule: the kernel is ACT-bound (softmax exp = 98304 elem/partition
= ~82us at 1.2GHz), so everything is organized as a software pipeline
that keeps the Activation engine streaming exps continuously:
  - heads 0,1 are projected stacked on partitions 0..127; their score
    matmuls (K=64) are emitted as adjacent row-tile pairs
    (tile_position (0,0)/(64,0)) so the PE can overlap them on HW
  - per q-block iteration: [PV h0,h1 of prev block || exp h2 prev],
    [scores h0,h1 || PV h2 + out-proj of prev], [scores h2 || V-proj
    or spare]
  - softmax reciprocal is broadcast across partitions with a tiny
    K=1 matmul into the unused partitions 64..127 of the PV PSUM bank
    (no DRAM bounce)
  - output projection + ReduceScatter run per 512-row q-block inside
    the pipeline; bias enters via a ones-row appended to outT
"""

import numpy as np
import ml_dtypes

B = 2
S = 2048
D = 768
H = 12
DK = 64
HPC = 3           # heads per core
HD = HPC * DK     # 192 head-feature columns per core
NCORES = 8
GROUP = 4         # cores per batch (reduce-scatter group)
QS = S // GROUP   # 512 output rows per core

_compiled = None


def _build(reps=1, collective=True):
    """Build the SPMD program. reps>1 emits the whole pipeline N times
    back-to-back (same inputs/outputs) — used only for timing, where
    (T_reps - T_1)/(reps-1) cancels the per-dispatch overhead.
    collective=False drops the final ReduceScatter (for TimelineSim)."""
    import concourse.mybir as mybir
    import concourse.tile as tile
    from concourse import bacc
    from concourse.bass import ts
    import concourse.bass as bass

    bf16 = mybir.dt.bfloat16
    f32 = mybir.dt.float32

    nc = bacc.Bacc(num_devices=NCORES)

    qt = nc.dram_tensor("qt", [D, S], bf16, kind="ExternalInput")
    kt = nc.dram_tensor("kt", [D, S], bf16, kind="ExternalInput")
    vt = nc.dram_tensor("vt", [D, S], bf16, kind="ExternalInput")
    wq = nc.dram_tensor("wq", [D, HD], bf16, kind="ExternalInput")
    wk = nc.dram_tensor("wk", [D, HD], bf16, kind="ExternalInput")
    wv = nc.dram_tensor("wv", [D, HD], bf16, kind="ExternalInput")
    wo = nc.dram_tensor("wo", [HD + 1, D], bf16, kind="ExternalInput")
    bq = nc.dram_tensor("bq", [HD, 1], f32, kind="ExternalInput")
    bk = nc.dram_tensor("bk", [HD, 1], f32, kind="ExternalInput")
    bv = nc.dram_tensor("bv", [1, HD], f32, kind="ExternalInput")
    out_ext = nc.dram_tensor("out", [QS, D], bf16, kind="ExternalOutput")
    out_part = nc.dram_tensor("out_part", [S, D], bf16)
    out_rs = nc.dram_tensor("out_rs", [QS, D], bf16)

    RGROUPS = [list(range(g * GROUP, (g + 1) * GROUP))
               for g in range(NCORES // GROUP)]
    NC_ = D // 128      # 6 contraction chunks for the projections
    NKC = S // 128      # 16 kv chunks
    NQB = S // 512      # 4 q blocks
    VW = DK + 2         # 66-wide per-head V block: 64 dims + ones col + pad
    SCALE = float(1.0 / np.sqrt(DK))

    import contextlib

    with tile.TileContext(nc) as tc:
      with (tc.For_i(0, reps, 1) if reps > 1 else contextlib.nullcontext()):
       with contextlib.ExitStack() as ctx:
        consts = ctx.enter_context(tc.tile_pool(name="consts", bufs=1))
        acts = ctx.enter_context(tc.tile_pool(name="acts", bufs=1))

        # ---- load inputs, in consumption order ----
        # Two HWDGE queues (SP + Pool/gpsimd) so the ACT engine is never
        # burdened with DMA dispatch.
        dmae = [nc.sync, nc.gpsimd]
        ins_sb, w_sb, bias_sb = {}, {}, {}

        def load_w(name, t):
            sb = consts.tile([128, NC_, HD], bf16, tag=name)
            nc.gpsimd.dma_start(
                out=sb, in_=t[:, :].rearrange("(c p) n -> p c n", p=128))
            w_sb[name] = sb

        def load_bias(name, t):
            b0 = consts.tile([128, 1], f32, tag=name + "0")
            nc.sync.dma_start(out=b0, in_=t[0:128, :])
            b1 = consts.tile([HD - 128, 1], f32, tag=name + "1")
            nc.sync.dma_start(out=b1, in_=t[128:HD, :])
            bias_sb[name] = (b0, b1)

        def load_in(name, t, di=[0]):
            sb = consts.tile([128, NC_, S], bf16, tag=name)
            for c in range(NC_):
                dmae[di[0] % 2].dma_start(
                    out=sb[:, c, :], in_=t[c * 128:(c + 1) * 128, :])
                di[0] += 1
            ins_sb[name] = sb

        load_w("wk", wk)
        load_bias("bk", bk)
        load_in("kt", kt)
        load_w("wq", wq)
        load_bias("bq", bq)
        load_in("qt", qt)
        load_w("wv", wv)
        bv_bc = consts.tile([128, HD], f32, tag="bv")
        nc.sync.dma_start(
            out=bv_bc,
            in_=bass.AP(tensor=bv[:, :].tensor, offset=bv[:, :].offset,
                        ap=[[0, 128]] + bv[:, :].ap[1:]))
        load_in("vt", vt)
        wo0 = consts.tile([128, D], bf16, tag="wo0")
        nc.gpsimd.dma_start(out=wo0, in_=wo[0:128, :])
        wo1 = consts.tile([HD + 1 - 128, D], bf16, tag="wo1")
        nc.gpsimd.dma_start(out=wo1, in_=wo[128:HD + 1, :])
        ones_sb = consts.tile([1, 128], bf16, tag="ones")
        nc.vector.memset(ones_sb, 1.0)
        # Touch the exp table early so ACT's table DMA overlaps the loads.
        warm = consts.tile([1, 1], f32, tag="warm")
        nc.vector.memset(warm, 0.0)
        nc.scalar.activation(out=warm, in_=warm,
                             func=mybir.ActivationFunctionType.Exp)

        # ---- persistent SBUF activation tiles ----
        # heads 0,1 stacked [128, S]; head 2 as two [64, S] tiles
        qT0 = acts.tile([128, S], bf16, tag="qT0")
        kT0 = acts.tile([128, S], bf16, tag="kT0")
        qT1 = acts.tile([64, S], bf16, tag="qT1")
        kT1 = acts.tile([64, S], bf16, tag="kT1")
        qk0 = {"q": qT0, "k": kT0}
        qk1 = {"q": qT1, "k": kT1}
        outT0 = acts.tile([128, S], bf16, tag="outT0")
        outT1 = acts.tile([DK + 1, S], bf16, tag="outT1")
        nc.vector.memset(outT1[DK:DK + 1, :], 1.0)
        v_sb = acts.tile([128, NKC, HPC * VW], bf16, tag="v")
        for h in range(HPC):
            nc.vector.memset(v_sb[:, :, h * VW + DK:h * VW + DK + 1], 1.0)

        # PSUM budget (8 banks of [128,512]f32):
        #  sc0/sc1: [128,2,512] x1 each = 4 banks (h0/h1 score rounds,
        #           also k-projection accumulators)
        #  sc2:     [128,512] x2 = 2 banks (h2 scores, double buffered)
        #  pv:      [128,512] x2 = 2 banks (everything else: q/g1/V
        #           projections, PV accumulators, out-proj tiles)
        sc_pool = ctx.enter_context(
            tc.tile_pool(name="sc_psum", bufs=1, space="PSUM"))
        sc2_pool = ctx.enter_context(
            tc.tile_pool(name="sc2_psum", bufs=2, space="PSUM"))
        pv_pool = ctx.enter_context(
            tc.tile_pool(name="pv_psum", bufs=2, space="PSUM"))
        sm_pool = ctx.enter_context(tc.tile_pool(name="sm", bufs=1))
        nrm_pool = ctx.enter_context(tc.tile_pool(name="nrm", bufs=4))
        fo_pool = ctx.enter_context(tc.tile_pool(name="fo", bufs=2))

        expt = {}        # (h) -> current expt SBUF tile [128, NKC, 512]
        pvps = {}        # (h) -> current PV PSUM tile

        # ---- k/q g0 projection for one 512-col block into psum ----
        def emit_proj_block(ps, name, qb):
            x_sb = ins_sb[name + "t"]
            for c in range(NC_):
                nc.tensor.matmul(
                    ps, lhsT=w_sb["w" + name][:, c, 0:128],
                    rhs=x_sb[:, c, ts(qb, 512)],
                    start=(c == 0), stop=(c == NC_ - 1))

        def evac_proj(ps, name, qb):
            nc.vector.tensor_scalar_add(
                out=qk0[name][:, ts(qb, 512)], in0=ps,
                scalar1=bias_sb["b" + name][0])

        # ---- units: closures emitted by the pipeline zipper ----
        def u_qproj(qb):
            def emit():
                ps = pv_pool.tile([128, 512], f32, tag="pv")
                emit_proj_block(ps, "q", qb)
                evac_proj(ps, "q", qb)
            return emit

        def u_g1proj(qb):
            # head 2 q/k: M=64 pair on PE col groups 0-1 / 2-3
            def emit():
                ps = pv_pool.tile([128, 512], f32, tag="pv")
                for c in range(NC_):
                    nc.tensor.matmul(
                        ps[0:64, :], lhsT=w_sb["wq"][:, c, 128:192],
                        rhs=ins_sb["qt"][:, c, ts(qb, 512)],
                        start=(c == 0), stop=(c == NC_ - 1),
                        tile_position=(0, 0))
                    nc.tensor.matmul(
                        ps[64:128, :], lhsT=w_sb["wk"][:, c, 128:192],
                        rhs=ins_sb["kt"][:, c, ts(qb, 512)],
                        start=(c == 0), stop=(c == NC_ - 1),
                        tile_position=(0, 64))
                nc.vector.tensor_scalar_add(
                    out=qk1["q"][:, ts(qb, 512)], in0=ps[0:64, :],
                    scalar1=bias_sb["bq"][1])
                nc.vector.tensor_scalar_add(
                    out=qk1["k"][:, ts(qb, 512)], in0=ps[64:128, :],
                    scalar1=bias_sb["bk"][1])
            return emit

        def u_vproj(st):
            def emit():
                ps = pv_pool.tile([128, 512], f32, tag="pv")
                for c in range(NC_):
                    nc.tensor.matmul(
                        ps[:, 0:HD], lhsT=ins_sb["vt"][:, c, ts(st, 128)],
                        rhs=w_sb["wv"][:, c, :],
                        start=(c == 0), stop=(c == NC_ - 1))
                for h in range(HPC):
                    nc.vector.tensor_add(
                        v_sb[:, st, h * VW:h * VW + DK],
                        ps[:, ts(h, 64)], bv_bc[:, ts(h, 64)])
            return emit

        def u_sc01(qb, r):
            # one round: kv chunks {2r, 2r+1} for heads 0+1 as adjacent
            # row-tile pairs, then the two exps
            def emit():
                if r == 0:
                    expt[0] = sm_pool.tile([128, NKC, 512], bf16, tag="e0", name="e0")
                    expt[1] = sm_pool.tile([128, NKC, 512], bf16, tag="e1", name="e1")
                ps = {0: sc_pool.tile([128, 2, 512], f32, tag="sc0", name="sc0"),
                      1: sc_pool.tile([128, 2, 512], f32, tag="sc1", name="sc1")}
                for j in (0, 1):
                    for h in (0, 1):
                        nc.tensor.matmul(
                            ps[h][:, j, :],
                            lhsT=qk0["k"][ts(h, 64), ts(2 * r + j, 128)],
                            rhs=qk0["q"][ts(h, 64), ts(qb, 512)],
                            start=True, stop=True)
                for h in (0, 1):
                    nc.scalar.activation(
                        out=expt[h][:, 2 * r:2 * r + 2, :],
                        in_=ps[h][:, 0:2, :],
                        func=mybir.ActivationFunctionType.Exp, scale=SCALE)
            return emit

        def u_sc2(qb, cp):
            # head 2, kv chunks {2cp, 2cp+1}, serial M=128 scores
            def emit():
                if cp == 0:
                    expt[2] = sm_pool.tile([128, NKC, 512], bf16, tag="e2", name="e2")
                for j in (0, 1):
                    c = 2 * cp + j
                    ps = sc2_pool.tile([128, 512], f32, tag="sc2")
                    nc.tensor.matmul(
                        ps, lhsT=qk1["k"][:, ts(c, 128)],
                        rhs=qk1["q"][:, ts(qb, 512)], start=True, stop=True)
                    nc.scalar.activation(
                        out=expt[2][:, c, :], in_=ps,
                        func=mybir.ActivationFunctionType.Exp, scale=SCALE)
            return emit

        def u_pv(h, qb):
            def emit():
                ps = pv_pool.tile([128, 512], f32, tag="pv")
                pvps[h] = ps
                for kc in range(NKC):
                    nc.tensor.matmul(
                        ps[0:DK + 1, :],
                        lhsT=v_sb[:, kc, h * VW:h * VW + DK + 1],
                        rhs=expt[h][:, kc, :],
                        start=(kc == 0), stop=(kc == NKC - 1))
            return emit

        def u_norm(h, qb):
            # reciprocal of the ones-row, broadcast across partitions
            # 64..127 of the same PSUM bank via a K=1 matmul, multiply.
            def emit():
                ps = pvps[h]
                recip = nrm_pool.tile([1, 512], bf16, tag="recip")
                with nc.allow_low_precision(reason="softmax recip in bf16"):
                    nc.vector.reciprocal(recip, ps[DK:DK + 1, :])
                nc.tensor.matmul(
                    ps[64:128, :], lhsT=ones_sb[:, 0:64], rhs=recip,
                    start=True, stop=True, tile_position=(0, 64))
                dst = (outT0[ts(h, 64), ts(qb, 512)] if h < 2
                       else outT1[0:64, ts(qb, 512)])
                nc.vector.tensor_mul(dst, ps[0:DK, :], ps[64:128, :])
            return emit

        ot_tile = {}

        def u_oproj(qb, sq):
            # output projection for q sub-tile qb*4+sq, N split 512+256
            def emit():
                if sq == 0:
                    ot_tile[qb] = fo_pool.tile([128, 4, D], bf16, tag="ot",
                                               name="ot")
                qt_ = qb * 4 + sq
                for noff, nsz in ((0, 512), (512, 256)):
                    ps = pv_pool.tile([128, 512], f32, tag="pv")
                    nc.tensor.matmul(
                        ps[:, 0:nsz], lhsT=outT0[:, ts(qt_, 128)],
                        rhs=wo0[:, noff:noff + nsz], start=True, stop=False)
                    nc.tensor.matmul(
                        ps[:, 0:nsz], lhsT=outT1[:, ts(qt_, 128)],
                        rhs=wo1[:, noff:noff + nsz], start=False, stop=True)
                    nc.vector.tensor_copy(
                        out=ot_tile[qb][:, sq, noff:noff + nsz],
                        in_=ps[:, 0:nsz])
            return emit

        def u_out(qb):
            def emit():
                nc.sync.dma_start(
                    out=out_part[:, :].rearrange(
                        "(g t p) d -> g p t d", p=128, t=4)[qb],
                    in_=ot_tile[qb])
                if collective:
                    nc.gpsimd.collective_compute(
                        "ReduceScatter", mybir.AluOpType.add,
                        replica_groups=RGROUPS,
                        ins=[out_part[ts(qb, 512), :]],
                        outs=[out_rs[ts(qb, 128), :]])
                nc.sync.dma_start(out=out_ext[ts(qb, 128), :],
                                  in_=(out_rs if collective
                                       else out_part)[ts(qb, 128), :])
            return emit

        def zip_emit(primary, secondary):
            """Interleave unit lists: primary paces (ACT-gated), secondary
            fills PE slack. Emits p0 s0 p1 s1 ... with leftovers appended."""
            n = max(len(primary), len(secondary))
            for i in range(n):
                if i < len(primary):
                    primary[i]()
                if i < len(secondary):
                    secondary[i]()

        # ---- prologue: k projection (both k-qb pairs share one sc tile
        # via the 2-chunk slots), then q block 0 ----
        for pair, tag in ((0, "sc0"), (1, "sc1")):
            ps = sc_pool.tile([128, 2, 512], f32, tag=tag)
            for j in (0, 1):
                qb = 2 * pair + j
                emit_proj_block(ps[:, j, :], "k", qb)
                evac_proj(ps[:, j, :], "k", qb)
        ps = pv_pool.tile([128, 512], f32, tag="pv")
        emit_proj_block(ps, "q", 0)
        evac_proj(ps, "q", 0)

        # ---- pipeline over q blocks ----
        # iteration qb emits: [PV h0,h1 of qb-1] then
        # [sc01(qb) || pv2(qb-1)+oproj(qb-1)+out(qb-1)] then
        # [sc2(qb) || spare work (projections on qb==0)]
        for it in range(NQB + 1):
            qb, pq = it, it - 1
            if pq >= 0:
                zip_emit([u_pv(0, pq), u_norm(0, pq),
                          u_pv(1, pq), u_norm(1, pq)], [])
            drain_b = ([u_pv(2, pq), u_norm(2, pq)] +
                       [u_oproj(pq, sq) for sq in range(4)] +
                       [u_out(pq)]) if pq >= 0 else []
            if it == 0:
                # g1 must precede sc2 units (h2 scores read its output)
                spare_b = ([u_qproj(1), u_qproj(2), u_qproj(3)] +
                           [u_g1proj(qb_) for qb_ in range(NQB)])
                spare_c = [u_vproj(st) for st in range(NKC)]
            else:
                spare_b, spare_c = drain_b, []
            if it < NQB:
                zip_emit([u_sc01(qb, r) for r in range(8)], spare_b)
                zip_emit([u_sc2(qb, cp) for cp in range(8)], spare_c)
            else:
                for u in drain_b:
                    u()

    nc.compile()
    return nc


def _get_compiled():
    global _compiled
    if _compiled is None:
        _compiled = _build()
    return _compiled


def make_in_maps(q, k, v, Wq, bq, Wk, bk, Wv, bv, Wo, bo):
    bf = ml_dtypes.bfloat16
    in_maps = []
    for c in range(NCORES):
        b = c // GROUP
        g = c % GROUP
        cols = slice(g * HD, (g + 1) * HD)   # head-feature columns
        wo_aug = np.empty((HD + 1, D), np.float32)
        wo_aug[:HD] = Wo.T[cols.start:cols.stop, :]
        wo_aug[HD] = bo / GROUP              # summed GROUP times by the RS
        in_maps.append({
            "qt": np.ascontiguousarray(q[b].T).astype(bf),
            "kt": np.ascontiguousarray(k[b].T).astype(bf),
            "vt": np.ascontiguousarray(v[b].T).astype(bf),
            "wq": np.ascontiguousarray(Wq.T[:, cols]).astype(bf),
            "wk": np.ascontiguousarray(Wk.T[:, cols]).astype(bf),
            "wv": np.ascontiguousarray(Wv.T[:, cols]).astype(bf),
            "wo": wo_aug.astype(bf),
            "bq": np.ascontiguousarray(bq[cols].reshape(HD, 1)).astype(np.float32),
            "bk": np.ascontiguousarray(bk[cols].reshape(HD, 1)).astype(np.float32),
            "bv": np.ascontiguousarray(bv[cols].reshape(1, HD)).astype(np.float32),
        })
    return in_maps


def kernel(q, k, v, Wq, bq, Wk, bk, Wv, bv, Wo, bo):
    from concourse.bass_utils import run_bass_kernel_spmd

    q = np.asarray(q, np.float32)
    k = np.asarray(k, np.float32)
    v = np.asarray(v, np.float32)
    nc = _get_compiled()
    in_maps = make_in_maps(q, k, v,
                           np.asarray(Wq, np.float32), np.asarray(bq, np.float32),
                           np.asarray(Wk, np.float32), np.asarray(bk, np.float32),
                           np.asarray(Wv, np.float32), np.asarray(bv, np.float32),
                           np.asarray(Wo, np.float32), np.asarray(bo, np.float32))
    res = run_bass_kernel_spmd(nc, in_maps, list(range(NCORES))).results
    out = np.empty((B, S, D), np.float32)
    for c in range(NCORES):
        b = c // GROUP
        j = c % GROUP
        # chunked reduce-scatter: chunk g of core (b, j) holds batch-b
        # rows [512*g + 128*j, 512*g + 128*j + 128)
        chunks = res[c]["out"].reshape(GROUP, 128, D)
        for g in range(GROUP):
            out[b, 512 * g + 128 * j:512 * g + 128 * j + 128, :] = chunks[g]
    return out
